# revision 20
# baseline (speedup 1.0000x reference)
"""Block-local self-attention (BLOCK=128, 3-block windows + global token) on 8
Trainium2 NeuronCores.

Sharding: batch*heads = 32 (n,h) pairs -> 4 pairs per core, no cross-core comms.

Design notes (v3):
  - additive mask is applied as the ACT exp's per-partition *bias* operand
    (it only varies along k), so the QK contraction is exactly 64 and the
    scores matmuls are ROW-TILED: even k-block stationaries load into PE
    rows 0-63, odd into rows 64-127 (Q^T duplicated into partitions 64-127),
    and the two 384-wide score matmuls of a block pair run concurrently.
  - PV is transposed: the V' slab ([128 k, 65] = V|ones) is the stationary,
    shared by the windows that use the slab.  Adjacent windows living in the
    same PSUM bank are computed by ONE matmul (N=256/384), so PV averages
    N=256 per instruction.  Output is ctxT [65, q], 4 windows per PSUM bank;
    row 64 accumulates the softmax denominator.  Host normalizes.
  - PSUM has_written semantics: start=True clears the whole 2KB bank, so
    only the chronologically-first matmul of each group bank carries it.
  - global-token slot: e0[q] = exp(q.k0*scale + m0) for all q via a width-1
    stationary (k0 column) as 8 concurrent 512-col matmuls into two PSUM
    partition groups, exp'd in bulk, then added to each window group as a
    contraction-1 rank-1 matmul (stationary = V'[0] row at partitions 0/64)
    streaming 256 e0 columns; the ones column folds e0 into the denominator.
  - the single global query row (token 0 attends to all keys) is computed on
    the host in numpy (1/4000 of the output) and overwrites row 0.

Output is (pair, group-pair, 65, 1024) with q linear inside a group; the
host divides rows 0..63 by denominator row 64 and transposes.
"""

import numpy as np
import ml_dtypes

N, H, T, D = 2, 16, 4000, 64
BLOCK = 128
TP = 4096            # padded token count (32 blocks)
W = 32               # number of 128-blocks
NCORES = 8
PAIRS = N * H        # 32
PPC = PAIRS // NCORES  # pairs per core
NEG = -30000.0
SCALE = 1.0 / np.sqrt(np.float32(D))

_prog_cache = {}


def _qlo(j):
    return min(max(j - 1, 0), W - 3)


def _build_program():
    if "nc" in _prog_cache:
        return _prog_cache["nc"]

    import concourse.bacc as bacc
    import concourse.mybir as mybir
    from concourse import tile

    dt = mybir.dt
    EXP = mybir.ActivationFunctionType.Exp

    nc = bacc.Bacc("TRN2", target_bir_lowering=False, debug=False,
                   num_devices=NCORES)
    # Q^T*scale, duplicated into both partition halves: [128, TP]
    qtc_d = nc.dram_tensor("qtc", [PPC, 128, TP], dt.bfloat16,
                           kind="ExternalInput").ap()
    # K^T packed: block j at partitions 64*(j%2), cols (j//2)*128
    kte_d = nc.dram_tensor("kte", [PPC, 128, (W // 2) * 128], dt.bfloat16,
                           kind="ExternalInput").ap()
    # V' = [V | 1]: vp[p, j*65+f] = V'[j*128+p, f]
    vp_d = nc.dram_tensor("vp", [PPC, 128, W * 65], dt.bfloat16,
                          kind="ExternalInput").ap()
    # col j<32: additive mask for k-block j (per partition); col 32: m0
    maskc_d = nc.dram_tensor("maskc", [PPC, 128, 33], dt.float32,
                             kind="ExternalInput").ap()
    # K[0] column (rows 0..63), width 1
    k0rep_d = nc.dram_tensor("k0rep", [PPC, 128, 1], dt.bfloat16,
                             kind="ExternalInput").ap()
    # V'[0] row at partitions 0 and 64
    v0r_d = nc.dram_tensor("v0r", [PPC, 128, 65], dt.bfloat16,
                           kind="ExternalInput").ap()
    out_d = nc.dram_tensor("out", [PPC, 4, 65, 1024], dt.bfloat16,
                           kind="ExternalOutput").ap()

    with tile.TileContext(nc) as tc:
        with (
            tc.tile_pool(name="qtc", bufs=2) as qtc_pool,
            tc.tile_pool(name="kte", bufs=2) as kte_pool,
            tc.tile_pool(name="vp", bufs=2) as vp_pool,
            tc.tile_pool(name="small", bufs=2) as small_pool,
            tc.tile_pool(name="ex", bufs=4) as ex_pool,
            tc.tile_pool(name="e0rep", bufs=2) as e0rep_pool,
            tc.tile_pool(name="outp", bufs=3) as out_pool,
            tc.tile_pool(name="sc", bufs=2, space="PSUM") as sc_pool,
            tc.tile_pool(name="ctx", bufs=3, space="PSUM") as ctx_pool,
        ):
            def load_pair(p):
                # spread the 2.1MB of per-pair loads across all 3 DMA rings;
                # small tensors first so e0 isn't blocked behind bulk loads.
                # pair 0 is on the critical path: split its bulk loads into
                # column chunks so the first QK batches can start while the
                # tail still streams in.
                nchunk = 2 if p == 0 else 1
                maskc_t = small_pool.tile([128, 33], dt.float32, tag="maskc",
                                          name=f"maskc_{p}")
                nc.gpsimd.dma_start(maskc_t[:], maskc_d[p])
                k0_t = small_pool.tile([128, 1], dt.bfloat16, tag="k0",
                                       name=f"k0_{p}")
                nc.gpsimd.dma_start(k0_t[:], k0rep_d[p])
                v0_t = small_pool.tile([128, 65], dt.bfloat16, tag="v0",
                                       name=f"v0_{p}")
                nc.gpsimd.dma_start(v0_t[:], v0r_d[p])
                kte_t = kte_pool.tile([128, (W // 2) * 128], dt.bfloat16,
                                      tag="kte", name=f"kte_{p}")
                qtc_t = qtc_pool.tile([128, TP], dt.bfloat16, tag="qtc",
                                      name=f"qtc_{p}")
                KW = (W // 2) * 128
                for ch in range(nchunk):
                    ks = slice(ch * KW // nchunk, (ch + 1) * KW // nchunk)
                    qs = slice(ch * TP // nchunk, (ch + 1) * TP // nchunk)
                    nc.sync.dma_start(kte_t[0:64, ks], kte_d[p, 0:64, ks])
                    nc.sync.dma_start(qtc_t[0:64, qs], qtc_d[p, 0:64, qs])
                    nc.gpsimd.dma_start(kte_t[64:128, ks],
                                        kte_d[p, 64:128, ks])
                    nc.gpsimd.dma_start(qtc_t[64:128, qs],
                                        qtc_d[p, 64:128, qs])
                vp_t = vp_pool.tile([128, W * 65], dt.bfloat16, tag="vp",
                                    name=f"vp_{p}")
                nc.scalar.dma_start(vp_t[:], vp_d[p])
                return qtc_t, kte_t, vp_t, maskc_t, k0_t, v0_t

            # PE warm-up: dense N=512 matmuls on memset data release the HAM
            # clock gate (K=8/8) while the first pair's inputs stream in.
            warm_sb = small_pool.tile([128, 1024], dt.bfloat16, tag="warm")
            nc.gpsimd.memset(warm_sb[:], 0.25)
            # dummy exp loads the ACT table set (~2.7us) during PE warm-up,
            # so the first real exp doesn't stall the pipeline
            nc.scalar.activation(warm_sb[:, 1008:1024], warm_sb[:, 0:16], EXP)
            warm_ps = sc_pool.tile([128, 1024], dt.float32, tag="sc",
                                   name="warm_ps")
            for r in range(50):
                nc.tensor.matmul(warm_ps[:, 0:512], warm_sb[:, 0:128],
                                 warm_sb[:, 0:512], start=True, stop=True)

            pending = {0: load_pair(0)}
            for p in range(PPC):
                qtc_t, kte_t, vp_t, maskc_t, k0_t, v0_t = pending.pop(p)

                # prefetch next pair's inputs
                if p + 1 < PPC:
                    pending[p + 1] = load_pair(p + 1)

                e0rep = e0rep_pool.tile([128, 2048], dt.bfloat16, tag="e0rep",
                                        name=f"e0rep_{p}")

                e0ps_tiles = {}

                def emit_e0_mm(i, p=p, qtc_t=qtc_t, k0_t=k0_t):
                    # e0: token-0 slot scores for q in [2048i, 2048i+2048).
                    # chunk c (q in [512c, 512c+512)) -> partition 64*(c%2),
                    # cols (c//2)*512 of e0rep; PSUM staging in sc-tag tiles.
                    ps = sc_pool.tile([128, 1024], dt.float32, tag="sc",
                                      name=f"e0ps{i}_{p}")
                    e0ps_tiles[i] = ps
                    for c in range(4 * i, 4 * i + 4):
                        col = ((c % 4) // 2) * 512
                        row = 64 * (c % 2)
                        nc.tensor.matmul(ps[row:row + 1, col:col + 512],
                                         k0_t[0:64, 0:1],
                                         qtc_t[0:64, c * 512:(c + 1) * 512],
                                         start=True, stop=True)

                def emit_e0_act(i, maskc_t=maskc_t, e0rep=e0rep):
                    nc.scalar.activation(
                        e0rep[:, i * 1024:(i + 1) * 1024],
                        e0ps_tiles.pop(i)[:],
                        EXP, bias=maskc_t[:, 32:33])

                ex_tiles = {}
                ctx_tiles = {}
                out_tiles = {}

                def e0slice(w0, width, e0rep=e0rep):
                    # e0 row for q in [w0*128, w0*128+width*128)
                    c = w0 // 4
                    row = 64 * (c % 2)
                    col = (c // 2) * 512 + (w0 % 4) * 128
                    return e0rep[row:row + 1, col:col + width * 128]

                def emit_qk(m, qtc_t=qtc_t, kte_t=kte_t, p=p):
                    sc = sc_pool.tile([128, 1024], dt.float32, tag="sc",
                                      name=f"sc_{p}_{m}")
                    for h in range(2):
                        j = 2 * m + h
                        lo = _qlo(j)
                        rows = slice(64 * h, 64 * h + 64)
                        nc.tensor.matmul(
                            sc[:, h * 512:h * 512 + 384],
                            kte_t[rows, m * 128:(m + 1) * 128],
                            qtc_t[rows, lo * 128:lo * 128 + 384],
                            start=True, stop=True)
                    return sc

                def emit_exp(m, sc, maskc_t=maskc_t, p=p):
                    ex = ex_pool.tile([128, 768], dt.bfloat16, tag="ex",
                                      name=f"ex_{p}_{m}")
                    if m in (0, W // 2 - 1):
                        # masks differ between the two slabs (token-0 row /
                        # tail padding): separate bias columns
                        for h in range(2):
                            nc.scalar.activation(
                                ex[:, h * 384:(h + 1) * 384],
                                sc[:, h * 512:h * 512 + 384],
                                EXP, bias=maskc_t[:, 2 * m + h:2 * m + h + 1])
                    else:
                        nc.scalar.activation(
                            ex[:].rearrange("p (b x) -> p b x", x=384),
                            sc[:].rearrange("p (b x) -> p b x",
                                            x=512)[:, :, 0:384],
                            EXP, bias=maskc_t[:, 2 * m:2 * m + 1])
                    ex_tiles[m] = ex

                def get_ctx(c, p=p):
                    if c not in ctx_tiles:
                        ctx_tiles[c] = ctx_pool.tile(
                            [128, 512], dt.float32, tag="ctx",
                            name=f"ctx_{p}_{c}")
                    return ctx_tiles[c]

                def emit_pv(m, vp_t=vp_t, p=p):
                    ex = ex_tiles.pop(m)
                    for h in range(2):
                        j = 2 * m + h
                        lo = _qlo(j)
                        vpj = vp_t[:, j * 65:(j + 1) * 65]
                        ws = [w for w in (j - 1, j, j + 1) if 0 <= w < W]
                        # split into runs of adjacent windows in one bank
                        runs = []
                        for w in ws:
                            if runs and w % 4 != 0 and runs[-1][-1] == w - 1:
                                runs[-1].append(w)
                            else:
                                runs.append([w])
                        for run in runs:
                            w0, ln = run[0], len(run)
                            ct = get_ctx(w0 // 4)
                            # start=True clears the whole bank's has_written
                            # bits: only the group's first matmul carries it
                            first = (w0 % 4 == 0) and (j == max(w0 - 1, 0))
                            last = (run[-1] == j + 1 == W - 1) or (
                                run[-1] % 4 == 3 and j == run[-1] + 1)
                            nc.tensor.matmul(
                                ct[0:65,
                                   (w0 % 4) * 128:(w0 % 4) * 128 + ln * 128],
                                vpj,
                                ex[:, h * 384 + (w0 - lo) * 128:
                                   h * 384 + (w0 - lo + ln) * 128],
                                start=first, stop=last,
                                skip_group_check=True)

                def emit_global(c, v0_t=v0_t):
                    ct = get_ctx(c)
                    row = 64 * (c % 2)
                    for half in range(2):
                        nc.tensor.matmul(
                            ct[0:65, half * 256:half * 256 + 256],
                            v0_t[row:row + 1, :],
                            e0slice(4 * c + 2 * half, 2),
                            start=False, stop=False,
                            skip_group_check=True)

                def emit_copy(c, p=p):
                    ct = ctx_tiles.pop(c)
                    if c % 2 == 0:
                        out_tiles[c // 2] = out_pool.tile(
                            [128, 1024], dt.bfloat16, tag="out",
                            name=f"out_{p}_{c // 2}")
                    ot = out_tiles[c // 2]
                    nc.vector.tensor_scalar_add(
                        ot[0:65, (c % 2) * 512:(c % 2) * 512 + 512],
                        ct[0:65, :], 0.0)
                    if c % 2 == 1:
                        nc.sync.dma_start(out_d[p, c // 2],
                                          out_tiles[c // 2][0:65, :])

                # software pipeline: QK two batches ahead, exp one ahead of
                # the PV consumption so the PE never waits on a fresh exp.
                def post_pv(mm):
                    if mm % 2 == 1:
                        emit_global((mm - 1) // 2)
                    if mm % 2 == 0 and mm >= 2:
                        emit_copy((mm - 2) // 2)

                # software pipeline: QK two batches ahead, exp one ahead
                # of the PV consumption so the PE never waits on fresh exps.
                scs = {0: emit_qk(0), 1: emit_qk(1)}
                for m in range(W // 2):
                    emit_exp(m, scs.pop(m))
                    if m + 2 < W // 2:
                        scs[m + 2] = emit_qk(m + 2)
                    if m == 0:
                        emit_e0_mm(0)
                    elif m == 1:
                        emit_e0_act(0)
                    elif m == 3:
                        emit_e0_mm(1)
                    elif m == 4:
                        emit_e0_act(1)
                    if m >= 1:
                        emit_pv(m - 1)
                        post_pv(m - 1)
                emit_pv(W // 2 - 1)
                post_pv(W // 2 - 1)
                emit_copy(W // 4 - 1)

    nc.compile()
    _prog_cache["nc"] = nc
    return nc


def _prep_core_inputs(q, k, v, mask):
    """q,k,v: (PAIRS, T, D) f32; mask: (N, T) f32.  Returns list of per-core
    input dicts (device layouts)."""
    bf16 = ml_dtypes.bfloat16
    in_maps = []
    for c in range(NCORES):
        qtc = np.zeros((PPC, 128, TP), np.float32)
        kte = np.zeros((PPC, 128, (W // 2) * 128), np.float32)
        vp = np.zeros((PPC, 128, W * 65), np.float32)
        maskc = np.full((PPC, 128, 33), NEG, np.float32)
        k0rep = np.zeros((PPC, 128, 1), np.float32)
        v0r = np.zeros((PPC, 128, 65), np.float32)
        for pp in range(PPC):
            pair = c * PPC + pp
            n = pair // H
            m_n = mask[n]
            QT = np.zeros((64, TP), np.float32)
            QT[:, :T] = q[pair].T * SCALE
            qtc[pp, 0:64] = QT
            qtc[pp, 64:128] = QT
            KT = np.zeros((64, TP), np.float32)
            KT[:, :T] = k[pair].T
            kb = KT.reshape(64, W, 128)
            kte[pp, 0:64] = kb[:, 0::2].reshape(64, -1)
            kte[pp, 64:128] = kb[:, 1::2].reshape(64, -1)
            mfull = np.full(TP, NEG, np.float32)
            mfull[:T] = m_n
            mfull[0] = NEG               # token-0 row served by global slot
            maskc[pp, :, 0:W] = mfull.reshape(W, 128).T
            maskc[pp, :, 32] = m_n[0]    # e0 bias
            k0rep[pp, 0:64, 0] = k[pair][0]
            Vp = np.zeros((TP, 65), np.float32)
            Vp[:T, :D] = v[pair]
            Vp[:, D] = 1.0
            vp[pp] = Vp.reshape(W, 128, 65).transpose(1, 0, 2).reshape(
                128, W * 65)
            v0r[pp, 0] = Vp[0]
            v0r[pp, 64] = Vp[0]
        in_maps.append({
            "qtc": qtc.astype(bf16),
            "kte": kte.astype(bf16),
            "vp": vp.astype(bf16),
            "maskc": maskc,
            "k0rep": k0rep.astype(bf16),
            "v0r": v0r.astype(bf16),
        })
    return in_maps


def _global_row(q, k, v, mask):
    """Token-0 query attends to all keys: one row per pair, in numpy."""
    out = np.empty((PAIRS, D), np.float32)
    for pair in range(PAIRS):
        n = pair // H
        s = (k[pair] @ (q[pair][0] * SCALE)) + mask[n]      # (T,)
        s = s - s.max()
        e = np.exp(s, dtype=np.float32)
        out[pair] = (e @ v[pair]) / e.sum()
    return out


def _unshard(results, grow):
    out = np.empty((PAIRS, T, D), np.float32)
    for c in range(NCORES):
        o = np.asarray(results[c]["out"], np.float32).reshape(
            PPC, 4, 65, 2, 512)
        o = o.transpose(0, 1, 3, 2, 4).reshape(PPC, 8, 65, 512)  # grp, f, q
        ctx = o[:, :, 0:64, :]
        den = o[:, :, 64:65, :]
        nrm = ctx / den                                  # (PPC, 8, 64, 512)
        nrm = nrm.transpose(0, 1, 3, 2).reshape(PPC, TP, D)[:, :T, :]
        nrm[:, 0, :] = grow[c * PPC:(c + 1) * PPC]
        out[c * PPC:(c + 1) * PPC] = nrm
    return out.reshape(N, H, T, D)


def _run(inputs, trace=False, tmpdir=None):
    from concourse.bass_utils import run_bass_kernel_spmd

    q = np.asarray(inputs["query_layer"], np.float32).reshape(PAIRS, T, D)
    k = np.asarray(inputs["key_layer"], np.float32).reshape(PAIRS, T, D)
    v = np.asarray(inputs["value_layer"], np.float32).reshape(PAIRS, T, D)
    mask = np.asarray(inputs["attention_mask"], np.float32).reshape(N, T)

    nc = _build_program()
    in_maps = _prep_core_inputs(q, k, v, mask)
    res = run_bass_kernel_spmd(nc, in_maps, list(range(NCORES)),
                               trace=trace, tmpdir=tmpdir)
    return _unshard(res.results, _global_row(q, k, v, mask)), res


def kernel(query_layer, key_layer, value_layer, attention_mask):
    out, _ = _run({
        "query_layer": query_layer,
        "key_layer": key_layer,
        "value_layer": value_layer,
        "attention_mask": attention_mask,
    })
    return out
